# revision 21
# baseline (speedup 1.0000x reference)
"""Fused multi-head attention for Trainium2 (Bass/Tile), 8-core SPMD.

Problem: B=2, H=16, S=4096, D=64, fp32 in/out, mask == all-ones.

Strategy (per core, 4 of the 32 (b,h) heads):
  * S^T orientation flash attention: keys on partitions, queries on the free
    dim, so no on-chip transposes are needed anywhere.
  * All matmul operands are bf16 (fp32 matmuls cost 2x on both LDWEIGHTS
    and MATMUL streaming; bf16 halves PE time and HBM traffic).
  * QK^T: lhsT = K^T tile [64, 128] bf16, rhs = Q^T block [64, 512] bf16
    -> S^T psum fp32 [128 keys, 1024 queries] (2-k-tile chunks). Even
    key-tiles use PE rows 0-63, odd rows 64-127: pairs run concurrently.
  * exp is split across TWO engines working concurrently on alternating
    2-k-tile chunks (1024 cols):
      - ScalarE: native activation Exp (scale=1/8 folded in), bf16 out.
      - DVE: Schraudolph fast-exp in one tensor_scalar: i16 = round(
        s*A + B) where A = 2^7*log2(e)/8, B = 2^7*(127-C).  The int16
        bit pattern IS the bf16 exp approximation (verified round-to-
        nearest on HW); the tile is bitcast to bf16 for P@V.
        End-to-end softmax error from the approximation ~1.0e-2 rel.
  * The QK pair for chunk c+3 is emitted IMMEDIATELY after exp(c) in
    program order: the stage PSUM slot freed by exp(c) is refilled after
    ~220ns (QK pair) instead of queueing behind ~430ns of PV matmuls,
    which otherwise starves the exp engines (PE is strict FIFO).
  * P@V: V' = [V, 1] (ones column accumulates the softmax denominator);
    each 128-key tile is split into two 64-key halves on PE rows 0-63 /
    64-127 running concurrently into two PSUM banks (full-row matmuls
    would block LDWEIGHTS preloading and expose drain at every matmul
    boundary).  ScalarE copy + DVE add merge the banks.  PV trails the
    exp chain by two chunks so PE work interleaves instead of bursting.
  * Normalization (divide by denominator row) and the final [65, S] ->
    [S, D] transpose happen host-side on the gathered outputs.

Inputs are pre-rearranged host-side (numpy) into the layouts the kernel
wants: Q^T duplicated onto both partition halves, K^T even/odd-packed, and
V' key-tile-major, all bf16. Input loads use SWDGE (gpsimd) dmas: large
HWDGE loads showed completion-semaphore races against pool-slot reuse.
"""

import numpy as np
import ml_dtypes

import concourse.mybir as mybir
import concourse.tile as tile
from concourse import bacc
from concourse.bass_utils import run_bass_kernel_spmd

B, H, S, D = 2, 16, 4096, 64
BH = B * H
N_CORES = 8
NH = BH // N_CORES          # heads per core
QB = 512                    # queries per q-block
N_QB = S // QB              # q-blocks per head
KT = S // 128               # 128-key tiles per head
CHUNK = 2                   # key-tiles per exp chunk (2 psum banks)
N_CH = KT // CHUNK          # chunks per q-block
QK_AHEAD = 2                # chunks of QK lead over the exp chain
                            # (with 3 stage bufs, lead 2 targets the slot
                            # freed by exp one chunk back; lead 3 would wait
                            # on the exp just issued and serialize the PE)
PV_TRAIL = 6                # chunks the PV chain trails the exp chain

F32 = mybir.dt.float32
BF16 = mybir.dt.bfloat16
I16 = mybir.dt.int16
NPBF16 = np.dtype(ml_dtypes.bfloat16)

# Schraudolph fast-exp constants (bf16 bit domain), 1/sqrt(D) folded in.
SCHRAUDOLPH_C = 0.0579
A_DVE = 128.0 / (np.log(2.0) * np.sqrt(float(D)))
B_DVE = 128.0 * (127.0 - SCHRAUDOLPH_C)

# ScalarE handles SCALAR_NUM of every SCALAR_DEN chunks (rest on DVE).
SCALAR_NUM, SCALAR_DEN = 16, 32

_cache = {}


def _build_program():
    nc = bacc.Bacc(num_swdge_queues=4)
    kt_in = nc.declare_dram_parameter("kt", [NH, 128, S // 2], BF16, isOutput=False)
    qt_in = nc.declare_dram_parameter("qt", [NH, 128, S], BF16, isOutput=False)
    v_in = nc.declare_dram_parameter("v", [NH, 128, KT * 65], BF16, isOutput=False)
    # Two unmerged PV halves per q-block; the host adds them (keeping the
    # on-chip merge off the exp engines' critical path).
    o_out = nc.declare_dram_parameter("o", [NH, 2, 65, S], F32, isOutput=True)

    with tile.TileContext(nc) as tc:
        with (
            tc.tile_pool(name="kt_p", bufs=2) as kt_pool,
            tc.tile_pool(name="qt_p", bufs=2) as qt_pool,
            tc.tile_pool(name="v_p", bufs=2) as v_pool,
            tc.tile_pool(name="pts_p", bufs=7) as pts_pool,
            tc.tile_pool(name="ptd_p", bufs=7) as ptd_pool,
            tc.tile_pool(name="osum_p", bufs=4) as osum_pool,
            tc.tile_pool(name="stage_p", bufs=3, space="PSUM") as stage_pool,
            tc.tile_pool(name="ot_p", bufs=2, space="PSUM") as ot_pool,
        ):
            class PVState:
                """P@V for one q-block, emitted chunk-by-chunk between the
                exp chunks."""

                def __init__(self, v_s, h, qb):
                    self.v_s, self.h, self.qb = v_s, h, qb
                    self.k = 0
                    self.queue = []
                    self.ot_a = ot_pool.tile([65, QB], F32, tag="ot")
                    self.ot_b = ot_pool.tile([65, QB], F32, tag="ot")

                def add_chunk(self, ptv, csz):
                    self.queue.append((ptv, csz))

                def emit_chunk(self):
                    ptv, csz = self.queue.pop(0)
                    for i in range(csz):
                        k = self.k + i
                        for half, ot in ((0, self.ot_a), (1, self.ot_b)):
                            nc.tensor.matmul(
                                ot[:, :],
                                self.v_s[64 * half:64 * half + 64,
                                         k * 65:(k + 1) * 65],
                                ptv[64 * half:64 * half + 64,
                                    i * QB:(i + 1) * QB],
                                start=(k == 0), stop=(k == KT - 1),
                                skip_group_check=True,
                            )
                    self.k += csz

                def finish_merge(self):
                    assert not self.queue and self.k == KT
                    for half, ot in ((0, self.ot_a), (1, self.ot_b)):
                        osum = osum_pool.tile([65, QB], F32, tag="osum")
                        nc.scalar.copy(osum[:, :], ot[:, :])
                        nc.sync.dma_start(
                            o_out[self.h, half, :,
                                  self.qb * QB:(self.qb + 1) * QB],
                            osum[:, :],
                        )

                def finish(self):
                    while self.queue:
                        self.emit_chunk()
                    self.finish_merge()

            def chunked_load(dst, src, widths):
                c0 = 0
                for w in widths:
                    nc.gpsimd.dma_start(dst[:, c0:c0 + w], src[:, c0:c0 + w])
                    c0 += w
                assert c0 == dst.shape[-1]

            def load_head(h):
                kt_s = kt_pool.tile([128, S // 2], BF16, tag="kt")
                qt_s = qt_pool.tile([128, S], BF16, tag="qt")
                v_s = v_pool.tile([128, KT * 65], BF16, tag="v")
                ld = nc.gpsimd.dma_start
                ld(kt_s[0:64, 0:128], kt_in[h][0:64, 0:128])        # key tile 0
                ld(kt_s[64:128, 0:128], kt_in[h][64:128, 0:128])    # key tile 1
                ld(qt_s[0:64, 0:QB], qt_in[h][0:64, 0:QB])
                ld(qt_s[64:128, 0:QB], qt_in[h][64:128, 0:QB])
                ld(kt_s[0:64, 128:256], kt_in[h][0:64, 128:256])    # tiles 2,3
                ld(kt_s[64:128, 128:256], kt_in[h][64:128, 128:256])
                chunked_load(
                    kt_s[:, 256:S // 2], kt_in[h][:, 256:S // 2], [448] * 4
                )
                chunked_load(v_s[:, :], v_in[h][:, :], [520, 520, 520, 520])
                chunked_load(qt_s[:, QB:2 * QB], qt_in[h][:, QB:2 * QB], [256, 256])
                chunked_load(
                    qt_s[:, 2 * QB:S], qt_in[h][:, 2 * QB:S], [QB] * 6
                )
                return kt_s, qt_s, v_s

            # One continuous chunk pipeline across all heads and q-blocks:
            # global chunk index G; QK runs QK_AHEAD chunks ahead of the exp
            # chain and PV trails PV_TRAIL chunks behind, with no pipeline
            # restart at q-block or head boundaries.
            N_G = NH * N_QB * N_CH
            head_tiles = {0: load_head(0)}
            stages = {}
            prev = None
            cur = None

            def emit_qk(G):
                h, r = divmod(G, N_QB * N_CH)
                qb, c = divmod(r, N_CH)
                if h not in head_tiles:
                    head_tiles[h] = load_head(h)
                    head_tiles.pop(h - 2, None)
                kt_s, qt_s, _ = head_tiles[h]
                st = stage_pool.tile([128, CHUNK * QB], F32, tag="stage")
                for i in range(CHUNK):
                    k = c * CHUNK + i
                    half = k % 2
                    blk = k // 2
                    nc.tensor.matmul(
                        st[:, i * QB:(i + 1) * QB],
                        kt_s[64 * half:64 * half + 64,
                             blk * 128:(blk + 1) * 128],
                        qt_s[64 * half:64 * half + 64,
                             qb * QB:(qb + 1) * QB],
                        start=True, stop=True,
                    )
                stages[G] = st

            for G in range(QK_AHEAD):
                emit_qk(G)
            for G in range(N_G):
                h, r = divmod(G, N_QB * N_CH)
                qb, c = divmod(r, N_CH)
                if c == 0:
                    prev = cur
                    cur = PVState(head_tiles[h][2], h, qb)
                st = stages.pop(G)
                use_scalar = (
                    ((G + 1) * SCALAR_NUM) // SCALAR_DEN
                    != (G * SCALAR_NUM) // SCALAR_DEN
                )
                if use_scalar:
                    pt = pts_pool.tile([128, CHUNK * QB], BF16, tag="pts")
                    nc.scalar.activation(
                        pt[:, :],
                        st[:, :],
                        mybir.ActivationFunctionType.Exp,
                        scale=1.0 / np.sqrt(float(D)),
                    )
                    ptv = pt[:, :]
                else:
                    pt = ptd_pool.tile([128, CHUNK * QB], I16, tag="ptd")
                    nc.vector.tensor_scalar(
                        pt[:, :], st[:, :],
                        float(A_DVE), float(B_DVE),
                        mybir.AluOpType.mult, mybir.AluOpType.add,
                    )
                    ptv = pt[:, :].bitcast(BF16)
                # Refill the stage slot freed by this exp with the G+2 QK
                # pair BEFORE any PV matmuls queue up (PE is strict FIFO).
                if G + QK_AHEAD < N_G:
                    emit_qk(G + QK_AHEAD)
                cur.add_chunk(ptv, CHUNK)
                # PE filler between exp chunks: one PV chunk per exp chunk,
                # draining the previous q-block's PV_TRAIL leftovers first;
                # the current q-block's PV trails the exp chain by PV_TRAIL
                # chunks, giving the previous q-block's output copies time
                # to free the ot banks before this q-block's first PV
                # matmul needs them.
                if c < PV_TRAIL:
                    if prev is not None:
                        prev.emit_chunk()
                        if c == PV_TRAIL - 1:
                            prev.finish_merge()
                            prev = None
                else:
                    cur.emit_chunk()
            prev = cur
            prev.finish()

    nc.compile()
    return nc


def _get_program():
    if "nc" not in _cache:
        _cache["nc"] = _build_program()
    return _cache["nc"]


def _pack_inputs(Q, K, V):
    """Host-side rearrangement into per-core device layouts (bf16)."""
    Qf = np.ascontiguousarray(Q.reshape(BH, S, D))
    Kf = np.ascontiguousarray(K.reshape(BH, S, D))
    Vf = np.ascontiguousarray(V.reshape(BH, S, D))

    # Q^T [BH, 64, S], duplicated onto both partition halves -> [BH, 128, S]
    QT = Qf.transpose(0, 2, 1)
    QTd = np.ascontiguousarray(
        np.concatenate([QT, QT], axis=1).astype(NPBF16)
    )

    # K^T [BH, 64, S] -> even key-tiles on partitions 0-63, odd on 64-127
    KTm = Kf.transpose(0, 2, 1).reshape(BH, D, KT, 128)
    KTpack = np.concatenate(
        [
            KTm[:, :, 0::2, :].reshape(BH, D, S // 2),
            KTm[:, :, 1::2, :].reshape(BH, D, S // 2),
        ],
        axis=1,
    ).astype(NPBF16)

    # V' = [V, ones]; key-tile-major layout [BH, 128, KT*65]
    Vp = np.concatenate([Vf, np.ones((BH, S, 1), np.float32)], axis=-1)
    Vb = np.ascontiguousarray(
        Vp.reshape(BH, KT, 128, 65)
        .transpose(0, 2, 1, 3)
        .reshape(BH, 128, KT * 65)
        .astype(NPBF16)
    )
    return KTpack, QTd, Vb


def kernel(Q, K, V, mask):
    assert Q.shape == (B, H, S, D)
    nc = _get_program()
    KTpack, QTd, Vb = _pack_inputs(
        np.asarray(Q, dtype=np.float32),
        np.asarray(K, dtype=np.float32),
        np.asarray(V, dtype=np.float32),
    )
    in_maps = []
    for c in range(N_CORES):
        sl = slice(c * NH, (c + 1) * NH)
        in_maps.append(
            {
                "kt": np.ascontiguousarray(KTpack[sl]),
                "qt": np.ascontiguousarray(QTd[sl]),
                "v": np.ascontiguousarray(Vb[sl]),
            }
        )
    res = run_bass_kernel_spmd(nc, in_maps, core_ids=list(range(N_CORES)))
    Oh = np.concatenate([r["o"] for r in res.results], axis=0)  # [BH, 2, 65, S]
    O = Oh[:, 0] + Oh[:, 1]                                    # [BH, 65, S]
    out = (O[:, :D, :] / O[:, D:D + 1, :]).transpose(0, 2, 1)  # [BH, S, D]
    return np.ascontiguousarray(out.reshape(B, H, S, D).astype(np.float32))


# revision 23
# speedup vs baseline: 1.0484x; 1.0484x over previous
"""Fused multi-head attention for Trainium2 (Bass/Tile), 8-core SPMD.

Problem: B=2, H=16, S=4096, D=64, fp32 in/out, mask == all-ones.

Strategy (per core, 4 of the 32 (b,h) heads):
  * S^T orientation flash attention: keys on partitions, queries on the free
    dim, so no on-chip transposes are needed anywhere.
  * All matmul operands are bf16 (fp32 matmuls cost 2x on both LDWEIGHTS
    and MATMUL streaming; bf16 halves PE time and HBM traffic).
  * QK^T: lhsT = K^T tile [64, 128] bf16, rhs = Q^T block [64, 512] bf16
    -> S^T psum fp32 [128 keys, 1024 queries] (2-k-tile chunks). Even
    key-tiles use PE rows 0-63, odd rows 64-127: pairs run concurrently.
  * exp is split across TWO engines working concurrently on alternating
    2-k-tile chunks (1024 cols):
      - ScalarE: native activation Exp (scale=1/8 folded in), bf16 out.
      - DVE: Schraudolph fast-exp in one tensor_scalar: i16 = round(
        s*A + B) where A = 2^7*log2(e)/8, B = 2^7*(127-C).  The int16
        bit pattern IS the bf16 exp approximation (verified round-to-
        nearest on HW); the tile is bitcast to bf16 for P@V.
        End-to-end softmax error from the approximation ~1.0e-2 rel.
  * One continuous chunk pipeline across all heads and q-blocks (global
    chunk index G, no pipeline restart at any boundary).  The QK pair
    for chunk G+2 is emitted IMMEDIATELY after exp(G) in program order:
    with 3 stage bufs it targets the PSUM slot freed one chunk back and
    refills it after ~220ns instead of queueing behind ~430ns of PV
    matmuls (PE is strict FIFO; lead 3 would wait on the exp just
    issued and serialize, lead 0 starves the exp engines).
  * P@V: V' = [V, 1] (ones column accumulates the softmax denominator);
    each 128-key tile is split into two 64-key halves on PE rows 0-63 /
    64-127 running concurrently into two PSUM banks (full-row matmuls
    would block LDWEIGHTS preloading and expose drain at every matmul
    boundary).  PV trails the exp chain by PV_TRAIL chunks, one PV chunk
    emitted per exp chunk, so PE work interleaves instead of bursting
    and the previous q-block's output copies have time to free the ot
    banks.  The two halves ship to DRAM UNMERGED via ScalarE copies
    (keeping a DVE add off the critical path); the host adds them.
  * Normalization (divide by denominator row) and the final [65, S] ->
    [S, D] transpose happen host-side on the gathered outputs.
  * This 48-slot-per-q-block PE schedule is streaming-optimal for the
    S^T formulation: QK needs 16 512-cycle slots (2 key-tiles each via
    row pairing), PV needs 32 (one per key-tile, halves paired), and the
    ones-column makes the denominator free; any M=64 quadrant repacking
    of PV pays the saved slots back as separate denominator reductions.

Inputs are pre-rearranged host-side (numpy) into the layouts the kernel
wants: Q^T duplicated onto both partition halves, K^T even/odd-packed, and
V' key-tile-major, all bf16. Input loads use SWDGE (gpsimd) dmas: large
HWDGE loads showed completion-semaphore races against pool-slot reuse.
"""

import numpy as np
import ml_dtypes

import concourse.mybir as mybir
import concourse.tile as tile
from concourse import bacc
from concourse.bass_utils import run_bass_kernel_spmd

B, H, S, D = 2, 16, 4096, 64
BH = B * H
N_CORES = 8
NH = BH // N_CORES          # heads per core
QB = 512                    # queries per q-block
N_QB = S // QB              # q-blocks per head
KT = S // 128               # 128-key tiles per head
CHUNK = 2                   # key-tiles per exp chunk (2 psum banks)
N_CH = KT // CHUNK          # chunks per q-block
QK_AHEAD = 2                # chunks of QK lead over the exp chain
                            # (with 3 stage bufs, lead 2 targets the slot
                            # freed by exp one chunk back; lead 3 would wait
                            # on the exp just issued and serialize the PE)
PV_TRAIL = 5                # chunks the PV chain trails the exp chain

F32 = mybir.dt.float32
BF16 = mybir.dt.bfloat16
I16 = mybir.dt.int16
NPBF16 = np.dtype(ml_dtypes.bfloat16)

# Schraudolph fast-exp constants (bf16 bit domain), 1/sqrt(D) folded in.
SCHRAUDOLPH_C = 0.0579
A_DVE = 128.0 / (np.log(2.0) * np.sqrt(float(D)))
B_DVE = 128.0 * (127.0 - SCHRAUDOLPH_C)

# ScalarE handles SCALAR_NUM of every SCALAR_DEN chunks (rest on DVE).
SCALAR_NUM, SCALAR_DEN = 16, 32

_cache = {}


def _build_program():
    nc = bacc.Bacc(num_swdge_queues=4)
    kt_in = nc.declare_dram_parameter("kt", [NH, 128, S // 2], BF16, isOutput=False)
    qt_in = nc.declare_dram_parameter("qt", [NH, 128, S], BF16, isOutput=False)
    v_in = nc.declare_dram_parameter("v", [NH, 128, KT * 65], BF16, isOutput=False)
    # Two unmerged PV halves per q-block; the host adds them (keeping the
    # on-chip merge off the exp engines' critical path).
    o_out = nc.declare_dram_parameter("o", [NH, 2, 65, S], F32, isOutput=True)

    with tile.TileContext(nc) as tc:
        with (
            tc.tile_pool(name="kt_p", bufs=2) as kt_pool,
            tc.tile_pool(name="qt_p", bufs=2) as qt_pool,
            tc.tile_pool(name="v_p", bufs=2) as v_pool,
            tc.tile_pool(name="pts_p", bufs=6) as pts_pool,
            tc.tile_pool(name="ptd_p", bufs=6) as ptd_pool,
            tc.tile_pool(name="osum_p", bufs=4) as osum_pool,
            tc.tile_pool(name="stage_p", bufs=3, space="PSUM") as stage_pool,
            tc.tile_pool(name="ot_p", bufs=2, space="PSUM") as ot_pool,
        ):
            class PVState:
                """P@V for one q-block, emitted chunk-by-chunk between the
                exp chunks."""

                def __init__(self, v_s, h, qb):
                    self.v_s, self.h, self.qb = v_s, h, qb
                    self.k = 0
                    self.queue = []
                    self.ot_a = ot_pool.tile([65, QB], F32, tag="ot")
                    self.ot_b = ot_pool.tile([65, QB], F32, tag="ot")

                def add_chunk(self, ptv, csz):
                    self.queue.append((ptv, csz))

                def emit_chunk(self):
                    ptv, csz = self.queue.pop(0)
                    for i in range(csz):
                        k = self.k + i
                        for half, ot in ((0, self.ot_a), (1, self.ot_b)):
                            nc.tensor.matmul(
                                ot[:, :],
                                self.v_s[64 * half:64 * half + 64,
                                         k * 65:(k + 1) * 65],
                                ptv[64 * half:64 * half + 64,
                                    i * QB:(i + 1) * QB],
                                start=(k == 0), stop=(k == KT - 1),
                                skip_group_check=True,
                            )
                    self.k += csz

                def finish_merge(self):
                    assert not self.queue and self.k == KT
                    for half, ot in ((0, self.ot_a), (1, self.ot_b)):
                        osum = osum_pool.tile([65, QB], F32, tag="osum")
                        nc.scalar.copy(osum[:, :], ot[:, :])
                        nc.sync.dma_start(
                            o_out[self.h, half, :,
                                  self.qb * QB:(self.qb + 1) * QB],
                            osum[:, :],
                        )

                def finish(self):
                    while self.queue:
                        self.emit_chunk()
                    self.finish_merge()

            def chunked_load(dst, src, widths):
                c0 = 0
                for w in widths:
                    nc.gpsimd.dma_start(dst[:, c0:c0 + w], src[:, c0:c0 + w])
                    c0 += w
                assert c0 == dst.shape[-1]

            def load_head(h):
                kt_s = kt_pool.tile([128, S // 2], BF16, tag="kt")
                qt_s = qt_pool.tile([128, S], BF16, tag="qt")
                v_s = v_pool.tile([128, KT * 65], BF16, tag="v")
                ld = nc.gpsimd.dma_start
                ld(kt_s[0:64, 0:128], kt_in[h][0:64, 0:128])        # key tile 0
                ld(kt_s[64:128, 0:128], kt_in[h][64:128, 0:128])    # key tile 1
                ld(qt_s[0:64, 0:QB], qt_in[h][0:64, 0:QB])
                ld(qt_s[64:128, 0:QB], qt_in[h][64:128, 0:QB])
                ld(kt_s[0:64, 128:256], kt_in[h][0:64, 128:256])    # tiles 2,3
                ld(kt_s[64:128, 128:256], kt_in[h][64:128, 128:256])
                chunked_load(
                    kt_s[:, 256:S // 2], kt_in[h][:, 256:S // 2], [448] * 4
                )
                chunked_load(v_s[:, :], v_in[h][:, :], [520, 520, 520, 520])
                chunked_load(qt_s[:, QB:2 * QB], qt_in[h][:, QB:2 * QB], [256, 256])
                chunked_load(
                    qt_s[:, 2 * QB:S], qt_in[h][:, 2 * QB:S], [QB] * 6
                )
                return kt_s, qt_s, v_s

            # One continuous chunk pipeline across all heads and q-blocks:
            # global chunk index G; QK runs QK_AHEAD chunks ahead of the exp
            # chain and PV trails PV_TRAIL chunks behind, with no pipeline
            # restart at q-block or head boundaries.
            N_G = NH * N_QB * N_CH
            head_tiles = {0: load_head(0)}
            stages = {}
            prev = None
            cur = None

            def emit_qk(G):
                h, r = divmod(G, N_QB * N_CH)
                qb, c = divmod(r, N_CH)
                if h not in head_tiles:
                    head_tiles[h] = load_head(h)
                    head_tiles.pop(h - 2, None)
                kt_s, qt_s, _ = head_tiles[h]
                st = stage_pool.tile([128, CHUNK * QB], F32, tag="stage")
                for i in range(CHUNK):
                    k = c * CHUNK + i
                    half = k % 2
                    blk = k // 2
                    nc.tensor.matmul(
                        st[:, i * QB:(i + 1) * QB],
                        kt_s[64 * half:64 * half + 64,
                             blk * 128:(blk + 1) * 128],
                        qt_s[64 * half:64 * half + 64,
                             qb * QB:(qb + 1) * QB],
                        start=True, stop=True,
                    )
                stages[G] = st

            for G in range(QK_AHEAD):
                emit_qk(G)
            for G in range(N_G):
                h, r = divmod(G, N_QB * N_CH)
                qb, c = divmod(r, N_CH)
                if c == 0:
                    prev = cur
                    cur = PVState(head_tiles[h][2], h, qb)
                st = stages.pop(G)
                use_scalar = (
                    ((G + 1) * SCALAR_NUM) // SCALAR_DEN
                    != (G * SCALAR_NUM) // SCALAR_DEN
                )
                if use_scalar:
                    pt = pts_pool.tile([128, CHUNK * QB], BF16, tag="pts")
                    nc.scalar.activation(
                        pt[:, :],
                        st[:, :],
                        mybir.ActivationFunctionType.Exp,
                        scale=1.0 / np.sqrt(float(D)),
                    )
                    ptv = pt[:, :]
                else:
                    pt = ptd_pool.tile([128, CHUNK * QB], I16, tag="ptd")
                    nc.vector.tensor_scalar(
                        pt[:, :], st[:, :],
                        float(A_DVE), float(B_DVE),
                        mybir.AluOpType.mult, mybir.AluOpType.add,
                    )
                    ptv = pt[:, :].bitcast(BF16)
                # Refill the stage slot freed by this exp with the G+2 QK
                # pair BEFORE any PV matmuls queue up (PE is strict FIFO).
                if G + QK_AHEAD < N_G:
                    emit_qk(G + QK_AHEAD)
                cur.add_chunk(ptv, CHUNK)
                # PE filler between exp chunks: one PV chunk per exp chunk,
                # draining the previous q-block's PV_TRAIL leftovers first;
                # the current q-block's PV trails the exp chain by PV_TRAIL
                # chunks, giving the previous q-block's output copies time
                # to free the ot banks before this q-block's first PV
                # matmul needs them.
                if c < PV_TRAIL:
                    if prev is not None:
                        prev.emit_chunk()
                        if c == PV_TRAIL - 1:
                            prev.finish_merge()
                            prev = None
                else:
                    cur.emit_chunk()
            prev = cur
            prev.finish()

    nc.compile()
    return nc


def _get_program():
    if "nc" not in _cache:
        _cache["nc"] = _build_program()
    return _cache["nc"]


def _pack_inputs(Q, K, V):
    """Host-side rearrangement into per-core device layouts (bf16)."""
    Qf = np.ascontiguousarray(Q.reshape(BH, S, D))
    Kf = np.ascontiguousarray(K.reshape(BH, S, D))
    Vf = np.ascontiguousarray(V.reshape(BH, S, D))

    # Q^T [BH, 64, S], duplicated onto both partition halves -> [BH, 128, S]
    QT = Qf.transpose(0, 2, 1)
    QTd = np.ascontiguousarray(
        np.concatenate([QT, QT], axis=1).astype(NPBF16)
    )

    # K^T [BH, 64, S] -> even key-tiles on partitions 0-63, odd on 64-127
    KTm = Kf.transpose(0, 2, 1).reshape(BH, D, KT, 128)
    KTpack = np.concatenate(
        [
            KTm[:, :, 0::2, :].reshape(BH, D, S // 2),
            KTm[:, :, 1::2, :].reshape(BH, D, S // 2),
        ],
        axis=1,
    ).astype(NPBF16)

    # V' = [V, ones]; key-tile-major layout [BH, 128, KT*65]
    Vp = np.concatenate([Vf, np.ones((BH, S, 1), np.float32)], axis=-1)
    Vb = np.ascontiguousarray(
        Vp.reshape(BH, KT, 128, 65)
        .transpose(0, 2, 1, 3)
        .reshape(BH, 128, KT * 65)
        .astype(NPBF16)
    )
    return KTpack, QTd, Vb


def kernel(Q, K, V, mask):
    assert Q.shape == (B, H, S, D)
    nc = _get_program()
    KTpack, QTd, Vb = _pack_inputs(
        np.asarray(Q, dtype=np.float32),
        np.asarray(K, dtype=np.float32),
        np.asarray(V, dtype=np.float32),
    )
    in_maps = []
    for c in range(N_CORES):
        sl = slice(c * NH, (c + 1) * NH)
        in_maps.append(
            {
                "kt": np.ascontiguousarray(KTpack[sl]),
                "qt": np.ascontiguousarray(QTd[sl]),
                "v": np.ascontiguousarray(Vb[sl]),
            }
        )
    res = run_bass_kernel_spmd(nc, in_maps, core_ids=list(range(N_CORES)))
    Oh = np.concatenate([r["o"] for r in res.results], axis=0)  # [BH, 2, 65, S]
    O = Oh[:, 0] + Oh[:, 1]                                    # [BH, 65, S]
    out = (O[:, :D, :] / O[:, D:D + 1, :]).transpose(0, 2, 1)  # [BH, S, D]
    return np.ascontiguousarray(out.reshape(B, H, S, D).astype(np.float32))
